# revision 23
# baseline (speedup 1.0000x reference)
"""Expert-parallel MoE FFN (SwiGLU) for 8 TRN2 NeuronCores.

Strategy: expert parallelism. Host sorts tokens by expert_id, pads each
expert's token group to a common capacity C (multiple of 128), and ships
core e: its expert's weights (bf16) + its tokens transposed [H, C] (bf16).
Each core runs a dense SwiGLU FFN for its expert in transposed layout
(features on partitions, tokens on the moving axis), so both weight
matrices are used directly as the stationary matmul operand with no
on-device transposes. Host unpermutes the per-core outputs.

Per-core compute: GU^T = Wgu^T-ish via out[M=feat,N=tok] = Wgu[k,feat].T @ xT[k,tok],
hidden = silu(G)*U (ACT + DVE, bf16), y^T = Wd-stationary matmul over F.
"""

import numpy as np
import ml_dtypes

import concourse.bass as bass  # noqa: F401
import concourse.tile as tile
from concourse import bacc, mybir
from concourse import bass_utils

H = 1024
F = 2048
F2 = 2 * F
E = 8
N_CORES = 8
P = 128
TOK_CHUNK = 512

BF16 = mybir.dt.bfloat16
F32 = mybir.dt.float32

_KERNEL_CACHE = {}


def _build(capacity: int):
    """Build + compile the per-core SPMD kernel for token capacity C."""
    KH = H // P      # 8  k-chunks for matmul 1
    KF = F // P      # 16 k-chunks for matmul 2
    NJ = F // P      # 16 gate/up feature-chunk pairs
    NHOUT = H // P   # 8  output row chunks

    nc = bacc.Bacc(
        "TRN2",
        target_bir_lowering=False,
        debug=False,
        num_devices=N_CORES,
    )
    xt_d = nc.dram_tensor("xt", [H, capacity], BF16, kind="ExternalInput").ap()
    wgu_d = nc.dram_tensor("wgu", [H, F2], BF16, kind="ExternalInput").ap()
    wd_d = nc.dram_tensor("wd", [F, H], BF16, kind="ExternalInput").ap()
    yt_d = nc.dram_tensor("yt", [H, capacity], BF16, kind="ExternalOutput").ap()

    # token chunks; ragged remainder first — a small chunk 0 shrinks the
    # head-critical DMA fill (x0 + wgu ft0) and buys the weight stream slack
    chunks = []
    rem = capacity % TOK_CHUNK
    t0 = 0
    if rem:
        chunks.append((0, rem))
        t0 = rem
    while t0 < capacity:
        chunks.append((t0, TOK_CHUNK))
        t0 += TOK_CHUNK

    FT = 512             # wgu sub-tile width in SBUF (DMA granularity)
    NFT = F2 // FT       # 4 sub-tiles per k-chunk
    JPF = FT // P        # 8 j-groups per sub-tile
    N_WARMUP = 24        # dummy matmuls to hold PE clock at 2.4 GHz during head

    with tile.TileContext(nc) as tc:
        with (
            tc.tile_pool(name="weights", bufs=1) as wpool,
            tc.tile_pool(name="xin", bufs=2) as xpool,
            tc.tile_pool(name="hid", bufs=2) as hpool,
            tc.tile_pool(name="gat", bufs=1) as gpool,
            tc.tile_pool(name="yout", bufs=3) as ypool,
            tc.tile_pool(name="ps1", bufs=2, space="PSUM") as ps1,
            tc.tile_pool(name="ps2", bufs=3, space="PSUM") as ps2,
        ):
            # DMA issue order matters: chunk-0 x first (1 MB), then wgu
            # ft-major; wd is not needed until chunk-0's down-projection.
            t0, nt = chunks[0]
            x_sb = []
            for k in range(KH):
                xt_sb = xpool.tile([P, nt], BF16, name=f"x{k}", tag=f"x{k}")
                nc.sync.dma_start(xt_sb[:], xt_d[k * P:(k + 1) * P, t0:t0 + nt])
                x_sb.append(xt_sb)


            # Weight DMAs issue from GpSimd: DMA issue costs ~620 ns of
            # sequencer time each, so a single engine can't issue the head's
            # x0+ft0 set fast enough — split issue across two engines.
            wgu_sb = [[None] * NFT for _ in range(KH)]
            for ft in range(NFT):
                for k in range(KH):
                    wt = wpool.tile(
                        [P, FT], BF16, name=f"wgu{k}_{ft}", tag=f"wgu{k}_{ft}"
                    )
                    nc.gpsimd.dma_start(
                        wt[:], wgu_d[k * P:(k + 1) * P, ft * FT:(ft + 1) * FT]
                    )
                    wgu_sb[k][ft] = wt
            wd_sb = [None] * KF

            for ci, (t0, nt) in enumerate(chunks):
                if ci == 0:
                    for f in range(KF):
                        wt = wpool.tile([P, H], BF16, name=f"wd{f}", tag=f"wd{f}")
                        nc.gpsimd.dma_start(wt[:], wd_d[f * P:(f + 1) * P, :])
                        wd_sb[f] = wt

                # gate pass: G_j = silu(sum_k wgu[k, j].T @ x[k]) into SBUF f32
                g_sb = []
                for j in range(NJ):
                    ft, jj = divmod(j, JPF)
                    g_ps = ps1.tile([P, nt], F32, name=f"g{j}", tag="g")
                    for k in range(KH):
                        nc.tensor.matmul(
                            g_ps[:],
                            wgu_sb[k][ft][:, jj * P:(jj + 1) * P],
                            x_sb[k][:],
                            start=(k == 0),
                            stop=(k == KH - 1),
                        )
                    gt = gpool.tile([P, nt], F32, name=f"gt{j}", tag=f"gt{j}")
                    nc.scalar.activation(
                        gt[:], g_ps[:], mybir.ActivationFunctionType.Silu
                    )
                    g_sb.append(gt)

                # Prefetch the next chunk's x: issued here (after this
                # chunk's gate-pass section) so the transfers don't compete
                # with the head weight fill for HBM.
                if ci + 1 < len(chunks):
                    t0n, ntn = chunks[ci + 1]
                    x_next = []
                    for k in range(KH):
                        xt_sb = xpool.tile([P, ntn], BF16, name=f"x{k}", tag=f"x{k}")
                        nc.sync.dma_start(
                            xt_sb[:], xt_d[k * P:(k + 1) * P, t0n:t0n + ntn]
                        )
                        x_next.append(xt_sb)

                # up pass: hidden_j = G_j * (sum_k wgu[k, 16+j].T @ x[k]) bf16
                h_sb = []
                for j in range(NJ):
                    ft, jj = divmod(NJ + j, JPF)
                    u_ps = ps1.tile([P, nt], F32, name=f"u{j}", tag="u")
                    for k in range(KH):
                        nc.tensor.matmul(
                            u_ps[:],
                            wgu_sb[k][ft][:, jj * P:(jj + 1) * P],
                            x_sb[k][:],
                            start=(k == 0),
                            stop=(k == KH - 1),
                        )
                    ht = hpool.tile([P, nt], BF16, name=f"h{j}", tag=f"h{j}")
                    nc.vector.tensor_mul(ht[:], g_sb[j][:], u_ps[:])
                    h_sb.append(ht)

                for hh in range(NHOUT):
                    y_ps = ps2.tile([P, nt], F32, name=f"y{hh}", tag="yp")
                    for f in range(KF):
                        nc.tensor.matmul(
                            y_ps[:],
                            wd_sb[f][:, hh * P:(hh + 1) * P],
                            h_sb[f][:],
                            start=(f == 0),
                            stop=(f == KF - 1),
                        )
                    y_sb = ypool.tile([P, nt], BF16, name="y", tag="y")
                    nc.scalar.copy(y_sb[:], y_ps[:])
                    nc.sync.dma_start(yt_d[hh * P:(hh + 1) * P, t0:t0 + nt], y_sb[:])

                x_sb = x_next

    nc.compile()
    return nc


def _get_kernel(capacity: int):
    if capacity not in _KERNEL_CACHE:
        _KERNEL_CACHE[capacity] = _build(capacity)
    return _KERNEL_CACHE[capacity]


def kernel(tokens, w_gate_up, w_down, expert_ids, _run_opts=None):
    tokens = np.asarray(tokens, dtype=np.float32)
    w_gate_up = np.asarray(w_gate_up, dtype=np.float32)
    w_down = np.asarray(w_down, dtype=np.float32)
    eids = np.asarray(expert_ids).astype(np.int64)
    n_tok = tokens.shape[0]

    counts = np.bincount(eids, minlength=E)
    capacity = int(max(P, -(-counts.max() // P) * P))

    order = np.argsort(eids, kind="stable")
    bf = ml_dtypes.bfloat16

    in_maps = []
    starts = np.zeros(E + 1, dtype=np.int64)
    np.cumsum(counts, out=starts[1:])
    for e in range(E):
        idx = order[starts[e]:starts[e + 1]]
        xe = np.zeros((capacity, H), dtype=np.float32)
        xe[: len(idx)] = tokens[idx]
        in_maps.append(
            {
                "xt": np.ascontiguousarray(xe.T).astype(bf),
                "wgu": w_gate_up[e].astype(bf),
                "wd": w_down[e].astype(bf),
            }
        )

    nc = _get_kernel(capacity)
    run_kwargs = dict(_run_opts or {})
    res = bass_utils.run_bass_kernel_spmd(
        nc, in_maps, core_ids=list(range(N_CORES)), **run_kwargs
    )

    out = np.zeros((n_tok, H), dtype=np.float32)
    for e in range(E):
        idx = order[starts[e]:starts[e + 1]]
        yt = res.results[e]["yt"]  # [H, capacity] bf16
        out[idx] = yt[:, : len(idx)].T.astype(np.float32)
    if run_kwargs.get("trace"):
        kernel.last_exec_time_ns = res.exec_time_ns
        kernel.last_results = res
    return out


# revision 24
# speedup vs baseline: 1.0439x; 1.0439x over previous
"""Expert-parallel MoE FFN (SwiGLU) for 8 TRN2 NeuronCores.

Strategy: expert parallelism. Host sorts tokens by expert_id, pads each
expert's token group to a common capacity C (multiple of 128), and ships
core e: its expert's weights (bf16) + its tokens transposed [H, C] (bf16).
Each core runs a dense SwiGLU FFN for its expert in transposed layout
(features on partitions, tokens on the moving axis), so both weight
matrices are used directly as the stationary matmul operand with no
on-device transposes. Host unpermutes the per-core outputs.

Per-core compute: GU^T = Wgu^T-ish via out[M=feat,N=tok] = Wgu[k,feat].T @ xT[k,tok],
hidden = silu(G)*U (ACT + DVE, bf16), y^T = Wd-stationary matmul over F.
"""

import numpy as np
import ml_dtypes

import concourse.bass as bass  # noqa: F401
import concourse.tile as tile
from concourse import bacc, mybir
from concourse import bass_utils

H = 1024
F = 2048
F2 = 2 * F
E = 8
N_CORES = 8
P = 128
TOK_CHUNK = 512

BF16 = mybir.dt.bfloat16
F32 = mybir.dt.float32

_KERNEL_CACHE = {}


def _build(capacity: int):
    """Build + compile the per-core SPMD kernel for token capacity C."""
    KH = H // P      # 8  k-chunks for matmul 1
    KF = F // P      # 16 k-chunks for matmul 2
    NJ = F // P      # 16 gate/up feature-chunk pairs
    NHOUT = H // P   # 8  output row chunks

    nc = bacc.Bacc(
        "TRN2",
        target_bir_lowering=False,
        debug=False,
        num_devices=N_CORES,
    )
    xt_d = nc.dram_tensor("xt", [H, capacity], BF16, kind="ExternalInput").ap()
    wgu_d = nc.dram_tensor("wgu", [H, F2], BF16, kind="ExternalInput").ap()
    wd_d = nc.dram_tensor("wd", [F, H], BF16, kind="ExternalInput").ap()
    yt_d = nc.dram_tensor("yt", [H, capacity], BF16, kind="ExternalOutput").ap()

    # token chunks (ragged remainder last: chunk 0 must be big enough that
    # its gate pass doesn't outrun the streaming weight DMA at the head)
    chunks = []
    t0 = 0
    while t0 < capacity:
        nt = min(TOK_CHUNK, capacity - t0)
        chunks.append((t0, nt))
        t0 += nt

    FT = 512             # wgu sub-tile width in SBUF (DMA granularity)
    NFT = F2 // FT       # 4 sub-tiles per k-chunk
    JPF = FT // P        # 8 j-groups per sub-tile
    N_WARMUP = 24        # dummy matmuls to hold PE clock at 2.4 GHz during head

    with tile.TileContext(nc) as tc:
        with (
            tc.tile_pool(name="weights", bufs=1) as wpool,
            tc.tile_pool(name="xin", bufs=2) as xpool,
            tc.tile_pool(name="hid", bufs=2) as hpool,
            tc.tile_pool(name="gat", bufs=1) as gpool,
            tc.tile_pool(name="yout", bufs=3) as ypool,
            tc.tile_pool(name="ps1", bufs=2, space="PSUM") as ps1,
            tc.tile_pool(name="ps2", bufs=3, space="PSUM") as ps2,
        ):
            # DMA issue order matters: chunk-0 x first (1 MB), then wgu
            # ft-major; wd is not needed until chunk-0's down-projection.
            t0, nt = chunks[0]
            x_sb = []
            for k in range(KH):
                xt_sb = xpool.tile([P, nt], BF16, name=f"x{k}", tag=f"x{k}")
                nc.sync.dma_start(xt_sb[:], xt_d[k * P:(k + 1) * P, t0:t0 + nt])
                x_sb.append(xt_sb)


            # Weight DMAs issue from GpSimd: DMA issue costs ~620 ns of
            # sequencer time each, so a single engine can't issue the head's
            # x0+ft0 set fast enough — split issue across two engines.
            wgu_sb = [[None] * NFT for _ in range(KH)]
            for ft in range(NFT):
                for k in range(KH):
                    wt = wpool.tile(
                        [P, FT], BF16, name=f"wgu{k}_{ft}", tag=f"wgu{k}_{ft}"
                    )
                    nc.gpsimd.dma_start(
                        wt[:], wgu_d[k * P:(k + 1) * P, ft * FT:(ft + 1) * FT]
                    )
                    wgu_sb[k][ft] = wt
            wd_sb = [None] * KF

            for ci, (t0, nt) in enumerate(chunks):
                if ci == 0:
                    for f in range(KF):
                        wt = wpool.tile([P, H], BF16, name=f"wd{f}", tag=f"wd{f}")
                        nc.gpsimd.dma_start(wt[:], wd_d[f * P:(f + 1) * P, :])
                        wd_sb[f] = wt

                # gate pass: G_j = silu(sum_k wgu[k, j].T @ x[k]) into SBUF f32
                g_sb = []
                for j in range(NJ):
                    ft, jj = divmod(j, JPF)
                    g_ps = ps1.tile([P, nt], F32, name=f"g{j}", tag="g")
                    for k in range(KH):
                        nc.tensor.matmul(
                            g_ps[:],
                            wgu_sb[k][ft][:, jj * P:(jj + 1) * P],
                            x_sb[k][:],
                            start=(k == 0),
                            stop=(k == KH - 1),
                        )
                    gt = gpool.tile([P, nt], F32, name=f"gt{j}", tag=f"gt{j}")
                    nc.scalar.activation(
                        gt[:], g_ps[:], mybir.ActivationFunctionType.Silu
                    )
                    g_sb.append(gt)

                # Prefetch the next chunk's x: issued here (after this
                # chunk's gate-pass section) so the transfers don't compete
                # with the head weight fill for HBM.
                if ci + 1 < len(chunks):
                    t0n, ntn = chunks[ci + 1]
                    x_next = []
                    for k in range(KH):
                        xt_sb = xpool.tile([P, ntn], BF16, name=f"x{k}", tag=f"x{k}")
                        nc.sync.dma_start(
                            xt_sb[:], xt_d[k * P:(k + 1) * P, t0n:t0n + ntn]
                        )
                        x_next.append(xt_sb)

                # up pass: hidden_j = G_j * (sum_k wgu[k, 16+j].T @ x[k]) bf16
                h_sb = []
                for j in range(NJ):
                    ft, jj = divmod(NJ + j, JPF)
                    u_ps = ps1.tile([P, nt], F32, name=f"u{j}", tag="u")
                    for k in range(KH):
                        nc.tensor.matmul(
                            u_ps[:],
                            wgu_sb[k][ft][:, jj * P:(jj + 1) * P],
                            x_sb[k][:],
                            start=(k == 0),
                            stop=(k == KH - 1),
                        )
                    ht = hpool.tile([P, nt], BF16, name=f"h{j}", tag=f"h{j}")
                    nc.vector.tensor_mul(ht[:], g_sb[j][:], u_ps[:])
                    h_sb.append(ht)

                for hh in range(NHOUT):
                    y_ps = ps2.tile([P, nt], F32, name=f"y{hh}", tag="yp")
                    for f in range(KF):
                        nc.tensor.matmul(
                            y_ps[:],
                            wd_sb[f][:, hh * P:(hh + 1) * P],
                            h_sb[f][:],
                            start=(f == 0),
                            stop=(f == KF - 1),
                        )
                    y_sb = ypool.tile([P, nt], BF16, name="y", tag="y")
                    nc.scalar.copy(y_sb[:], y_ps[:])
                    nc.sync.dma_start(yt_d[hh * P:(hh + 1) * P, t0:t0 + nt], y_sb[:])

                x_sb = x_next

    nc.compile()
    return nc


def _get_kernel(capacity: int):
    if capacity not in _KERNEL_CACHE:
        _KERNEL_CACHE[capacity] = _build(capacity)
    return _KERNEL_CACHE[capacity]


def kernel(tokens, w_gate_up, w_down, expert_ids, _run_opts=None):
    tokens = np.asarray(tokens, dtype=np.float32)
    w_gate_up = np.asarray(w_gate_up, dtype=np.float32)
    w_down = np.asarray(w_down, dtype=np.float32)
    eids = np.asarray(expert_ids).astype(np.int64)
    n_tok = tokens.shape[0]

    counts = np.bincount(eids, minlength=E)
    capacity = int(max(P, -(-counts.max() // P) * P))

    order = np.argsort(eids, kind="stable")
    bf = ml_dtypes.bfloat16

    in_maps = []
    starts = np.zeros(E + 1, dtype=np.int64)
    np.cumsum(counts, out=starts[1:])
    for e in range(E):
        idx = order[starts[e]:starts[e + 1]]
        xe = np.zeros((capacity, H), dtype=np.float32)
        xe[: len(idx)] = tokens[idx]
        in_maps.append(
            {
                "xt": np.ascontiguousarray(xe.T).astype(bf),
                "wgu": w_gate_up[e].astype(bf),
                "wd": w_down[e].astype(bf),
            }
        )

    nc = _get_kernel(capacity)
    run_kwargs = dict(_run_opts or {})
    res = bass_utils.run_bass_kernel_spmd(
        nc, in_maps, core_ids=list(range(N_CORES)), **run_kwargs
    )

    out = np.zeros((n_tok, H), dtype=np.float32)
    for e in range(E):
        idx = order[starts[e]:starts[e + 1]]
        yt = res.results[e]["yt"]  # [H, capacity] bf16
        out[idx] = yt[:, : len(idx)].T.astype(np.float32)
    if run_kwargs.get("trace"):
        kernel.last_exec_time_ns = res.exec_time_ns
        kernel.last_results = res
    return out
